# revision 8
# baseline (speedup 1.0000x reference)
"""Trainium2 Bass kernel for cosine-similarity contrastive loss (CosSimLoss).

reference:
    p = l2norm(pred).reshape(-1, C); t = l2norm(target).reshape(-1, C)
    logits = (p @ t.T) * e^0.5
    loss = mean(logsumexp(logits, axis=1) - diag(logits))

Strategy (8 NeuronCores, data parallel over N = B*L = 8192 rows):
  Each core receives only its 1024-row shard of pred AND of target. On
  device it computes row norms of both shards (DVE square-accum), scales
  its target shard by 8/||t|| and casts to fp8e4 (e4m3), then AllGathers
  the fp8 shards so every core holds the full normalized target (4 MiB
  instead of reading 16 MiB of fp32 HBM per core). logsumexp is
  permutation-invariant over target rows, so the gather is chunked and
  consumed in rank-permuted order for overlap.

  pred stays RAW in fp8; its 1/||p|| factor is folded into the per-
  partition scale of the exp activation. Both operands are transposed
  into contraction-on-partitions layout with uint16-viewed DMA xbar
  transposes (fp8 pairs packed in u16); the resulting byte-interleaved
  (j, i) -> c mapping is applied consistently to the stationary and
  moving APs, which is all fp8 DoubleRow matmuls require. DoubleRow
  contracts K=256 per instruction at 0.5 cycles/output-column (4x the
  bf16 baseline). Exp runs in-place on PSUM with a fused row-sum
  accumulator; since |cos|<=1 no max-subtraction pass is needed. The
  diagonal is computed exactly in fp32 from the local shards. Host sums
  per-core (lse - diag) partials and divides by N.
"""
import math

import numpy as np

import concourse.bacc as bacc
import concourse.mybir as mybir
import concourse.tile as tile
from concourse.bass_utils import run_bass_kernel_spmd

F32 = mybir.dt.float32
BF16 = mybir.dt.bfloat16
FP8 = mybir.dt.float8e4
U16 = mybir.dt.uint16
AF = mybir.ActivationFunctionType
ALU = mybir.AluOpType
AXIS = mybir.AxisListType
PM = mybir.MatmulPerfMode

TEMPERATURE = 0.5
SCALE = float(math.exp(TEMPERATURE))
FP8_GAIN = 8.0  # normalized target rows scaled by this before fp8 cast

# Full problem config (hardcoded per contest rules).
B, L, C = 4, 2048, 512
N_CORES = 8
N_TOTAL = B * L                  # 8192
M_LOCAL = N_TOTAL // N_CORES     # 1024 rows per core
MT = M_LOCAL // 128              # 8 output row tiles
QROWS = 512                      # rows per staging quad
NQ = M_LOCAL // QROWS            # 2 quads per shard
BLK = 2048                       # columns per psum block / exp drain
NB = N_TOTAL // BLK              # 4 blocks
JT = BLK // 512                  # psum 512-slices per block
KQ = C // 256                    # 2 u16-transpose chunks (256 c each)
CC_CHUNKS = NQ                   # all-gather chunks (QROWS rows/core each)


def build_nc():
    """Build + compile the per-core Bass program (SPMD: same NEFF, 8 cores)."""
    nc = bacc.Bacc("TRN2", target_bir_lowering=False, debug=False)
    pred = nc.dram_tensor("pred", [M_LOCAL, C], F32, kind="ExternalInput").ap()
    tgt = nc.dram_tensor("tgt", [M_LOCAL, C], F32, kind="ExternalInput").ap()
    out = nc.dram_tensor("out", [128, MT], F32, kind="ExternalOutput").ap()

    def quad_dram_ap(t2d, r0, rows):
        # DRAM rows [r0, r0+rows) viewed as [128 part, rows//128, c]
        return t2d[r0:r0 + rows, :].rearrange("(q p) c -> p q c", p=128)

    with tile.TileContext(nc) as tc:
        with (
            tc.tile_pool(name="dram", bufs=1, space="DRAM") as dram_pool,
            tc.tile_pool(name="ld", bufs=1) as ld_pool,
            tc.tile_pool(name="q8", bufs=1) as q8_pool,
            tc.tile_pool(name="sq", bufs=2) as sq_pool,
            tc.tile_pool(name="st", bufs=1) as stats_pool,
            tc.tile_pool(name="pT", bufs=1) as pT_pool,
            tc.tile_pool(name="tT", bufs=4) as tT_pool,
            tc.tile_pool(name="ps", bufs=2, space="PSUM") as psum_pool,
        ):
            # ---------------- Phase A: load shards, norms, diag -------------
            pq, tq = [], []
            for k in range(NQ):
                p_ld = ld_pool.tile([128, 4 * C], F32, name=f"pld{k}",
                                    tag=f"pld{k}")
                nc.sync.dma_start(p_ld[:].rearrange("p (q c) -> p q c", c=C),
                                  quad_dram_ap(pred, k * QROWS, QROWS))
                t_ld = ld_pool.tile([128, 4 * C], F32, name=f"tld{k}",
                                    tag=f"tld{k}")
                nc.sync.dma_start(t_ld[:].rearrange("p (q c) -> p q c", c=C),
                                  quad_dram_ap(tgt, k * QROWS, QROWS))
                pq.append(p_ld)
                tq.append(t_ld)

            sp = stats_pool.tile([128, MT], F32, name="sp", tag="sp")
            st = stats_pool.tile([128, MT], F32, name="st", tag="st")
            d0 = stats_pool.tile([128, MT], F32, name="d0", tag="d0")
            for k in range(NQ):
                for s in range(4):
                    col = k * 4 + s
                    a = pq[k][:, s * C:(s + 1) * C]
                    b = tq[k][:, s * C:(s + 1) * C]
                    sq1 = sq_pool.tile([128, C], F32, name="sqj", tag="sqj")
                    nc.vector.scalar_tensor_tensor(
                        sq1[:], a, 1.0, a, ALU.mult, ALU.mult,
                        accum_out=sp[:, col:col + 1])
                    sq2 = sq_pool.tile([128, C], F32, name="sqj", tag="sqj")
                    nc.vector.scalar_tensor_tensor(
                        sq2[:], b, 1.0, b, ALU.mult, ALU.mult,
                        accum_out=st[:, col:col + 1])
                    sq3 = sq_pool.tile([128, C], F32, name="sqj", tag="sqj")
                    nc.vector.scalar_tensor_tensor(
                        sq3[:], a, 1.0, b, ALU.mult, ALU.mult,
                        accum_out=d0[:, col:col + 1])

            # rsqrt via exp(-0.5 ln x); Ln/Exp grouped to avoid table swaps.
            lp = stats_pool.tile([128, MT], F32, name="lp", tag="lp")
            lt = stats_pool.tile([128, MT], F32, name="lt", tag="lt")
            nc.scalar.activation(lp[:], sp[:], AF.Ln)
            nc.scalar.activation(lt[:], st[:], AF.Ln)
            rp = stats_pool.tile([128, MT], F32, name="rp", tag="rp")
            rt = stats_pool.tile([128, MT], F32, name="rt", tag="rt")
            nc.scalar.activation(rp[:], lp[:], AF.Exp, scale=-0.5)
            nc.scalar.activation(rt[:], lt[:], AF.Exp, scale=-0.5)

            # exp scale per m-row, and exact scaled diagonal
            expsc = stats_pool.tile([128, MT], F32, name="expsc", tag="expsc")
            nc.vector.tensor_scalar_mul(expsc[:], rp[:], SCALE / FP8_GAIN)
            dtmp = stats_pool.tile([128, MT], F32, name="dtmp", tag="dtmp")
            nc.vector.tensor_mul(dtmp[:], d0[:], rt[:])
            diag = stats_pool.tile([128, MT], F32, name="diag", tag="diag")
            nc.vector.scalar_tensor_tensor(
                diag[:], dtmp[:], SCALE, rp[:], ALU.mult, ALU.mult)

            # ---------------- Phase B: fp8 cast, bounce, all-gather ---------
            # Target: normalized fp8, written COLUMN-PERMUTED so that after
            # the u16-packed xbar transpose, partition j's byte pair at
            # (2n, 2n+1) holds channels c = 256q + 128*{0,1} + j -- i.e. the
            # moving operand's (j, i) -> c map matches the planar weights.
            # Pred: raw bf16 bounce (exact u16 transpose), then a cheap DVE
            # cast into planar fp8 weight tiles.
            pb_dram = dram_pool.tile([M_LOCAL, C], BF16, name="pbd", tag="pbd")
            t8_loc = [dram_pool.tile([QROWS, C], FP8, name=f"t8l{k}",
                                     tag=f"t8l{k}") for k in range(CC_CHUNKS)]
            t8_gath = [dram_pool.tile([QROWS * N_CORES, C], FP8,
                                      name=f"t8g{k}", tag=f"t8g{k}")
                       for k in range(CC_CHUNKS)]
            for k in range(NQ):
                t8q = q8_pool.tile([128, 4 * C], FP8, name=f"t8q{k}",
                                   tag=f"t8q{k}")
                pbq = q8_pool.tile([128, 4 * C], BF16, name=f"pbq{k}",
                                   tag=f"pbq{k}")
                for s in range(4):
                    col = k * 4 + s
                    t8_out = t8q[:, s * C:(s + 1) * C].rearrange(
                        "p (q j i) -> p q i j", q=2, j=128, i=2)
                    t8_in = tq[k][:, s * C:(s + 1) * C].rearrange(
                        "p (q i j) -> p q i j", q=2, i=2, j=128)
                    nc.vector.tensor_scalar(
                        t8_out, t8_in,
                        rt[:, col:col + 1], FP8_GAIN, ALU.mult, ALU.mult)
                    nc.vector.tensor_scalar_mul(
                        pbq[:, s * C:(s + 1) * C], pq[k][:, s * C:(s + 1) * C],
                        1.0)
                nc.gpsimd.dma_start(quad_dram_ap(t8_loc[k], 0, QROWS),
                                    t8q[:].rearrange("p (q c) -> p q c", c=C))
                nc.gpsimd.collective_compute(
                    "AllGather", ALU.bypass,
                    replica_groups=[list(range(N_CORES))],
                    ins=[t8_loc[k].opt()], outs=[t8_gath[k].opt()])
                nc.sync.dma_start(quad_dram_ap(pb_dram, k * QROWS, QROWS),
                                  pbq[:].rearrange("p (q c) -> p q c", c=C))

            # ---------------- Phase C: pred transposes + planar fp8 --------
            pTb = []
            for kc in range(C // 128):  # 4 bf16 chunks of 128 channels
                pt = pT_pool.tile([128, M_LOCAL], BF16, name=f"pTb{kc}",
                                  tag=f"pTb{kc}")
                nc.sync.dma_start_transpose(
                    pt[:], pb_dram[0:M_LOCAL, kc * 128:(kc + 1) * 128])
                pTb.append(pt)
            pw = []
            for q in range(KQ):
                w = pT_pool.tile([128, 2 * M_LOCAL], FP8, name=f"pw{q}",
                                 tag=f"pw{q}")
                for i in range(2):
                    nc.vector.tensor_scalar_mul(
                        w[:, i * M_LOCAL:(i + 1) * M_LOCAL],
                        pTb[2 * q + i][:], 1.0)
                pw.append(w)

            # ---------------- Phase D: matmul + exp ------------------------
            sume = stats_pool.tile([128, MT * NB], F32, name="sume",
                                   tag="sume")
            blocks_per_chunk = NB // CC_CHUNKS
            for g in range(NB):
                cc_k = g // blocks_per_chunk
                goff = (g % blocks_per_chunk) * BLK
                t8u = t8_gath[cc_k].bitcast(U16)  # [QROWS*8, 256] u16
                tTg = []
                for q in range(KQ):
                    tt = tT_pool.tile([128, BLK], U16, name="tT", tag="tT")
                    nc.sync.dma_start_transpose(
                        tt[:], t8u[goff:goff + BLK, q * 128:(q + 1) * 128])
                    tTg.append(tt)
                for m in range(MT):
                    ps = psum_pool.tile([128, BLK], F32, name="ps", tag="ps")
                    for q in range(KQ):
                        w_ap = pw[q].rearrange(
                            "j (i m) -> j i m", i=2)[:, :, 128 * m:128 * (m + 1)]
                        for j in range(JT):
                            x_ap = tTg[q].bitcast(FP8)[
                                :, 1024 * j:1024 * (j + 1)].rearrange(
                                    "j (n i) -> j i n", i=2)
                            nc.tensor.matmul(
                                ps[:, j * 512:(j + 1) * 512], w_ap, x_ap,
                                start=(q == 0), stop=(q == KQ - 1),
                                perf_mode=PM.DoubleRow)
                    nc.scalar.activation(
                        ps[:], ps[:], AF.Exp, scale=expsc[:, m:m + 1],
                        accum_out=sume[:, m * NB + g:m * NB + g + 1])

            # ---------------- Phase E: lse - diag --------------------------
            rowsum = stats_pool.tile([128, MT], F32, name="rowsum",
                                     tag="rowsum")
            nc.vector.tensor_reduce(
                rowsum[:], sume[:].rearrange("p (m g) -> p m g", g=NB),
                axis=AXIS.X, op=ALU.add)
            lse = stats_pool.tile([128, MT], F32, name="lse", tag="lse")
            nc.scalar.activation(lse[:], rowsum[:], AF.Ln)
            losst = stats_pool.tile([128, MT], F32, name="losst", tag="losst")
            nc.vector.tensor_sub(losst[:], lse[:], diag[:])
            nc.sync.dma_start(out[:], losst[:])

    nc.compile()
    return nc


_NC_CACHE = {}


def _get_nc():
    key = (M_LOCAL, N_TOTAL, C)
    if key not in _NC_CACHE:
        _NC_CACHE[key] = build_nc()
    return _NC_CACHE[key]


def run_cores(pred2d, tgt2d, trace=False):
    """Run the SPMD program on cores 0..7; returns (partials [8,128,MT], res)."""
    nc = _get_nc()
    in_maps = []
    for ci in range(N_CORES):
        r0 = ci * M_LOCAL
        in_maps.append({
            "pred": np.ascontiguousarray(pred2d[r0:r0 + M_LOCAL]),
            "tgt": np.ascontiguousarray(tgt2d[r0:r0 + M_LOCAL]),
        })
    res = run_bass_kernel_spmd(nc, in_maps, list(range(N_CORES)), trace=trace)
    partials = np.stack([res.results[i]["out"] for i in range(N_CORES)])
    return partials, res


def kernel(pred, target):
    pred2d = np.asarray(pred, dtype=np.float32).reshape(-1, C)
    tgt2d = np.asarray(target, dtype=np.float32).reshape(-1, C)
    partials, _ = run_cores(pred2d, tgt2d)
    loss = partials.astype(np.float64).sum() / float(N_TOTAL)
    return np.float32(loss)


# revision 9
# speedup vs baseline: 1.3179x; 1.3179x over previous
"""Trainium2 Bass kernel for cosine-similarity contrastive loss (CosSimLoss).

reference:
    p = l2norm(pred).reshape(-1, C); t = l2norm(target).reshape(-1, C)
    logits = (p @ t.T) * e^0.5
    loss = mean(logsumexp(logits, axis=1) - diag(logits))

Strategy (8 NeuronCores, data parallel over N = B*L = 8192 rows of pred):
  Each core gets a 1024-row shard of pred plus the full target. Per
  2048-row target block it computes row norms (DVE square-accum), scales
  by 8/||t|| and casts to fp8e4 with a column-PERMUTED write so that the
  uint16-packed DMA xbar transpose lands the moving operand with the
  byte-interleaved (j, i) -> c map c = 256q + 128i + j. pred stays RAW:
  it is cast to bf16, transposed exactly via the 2-byte xbar, then cast
  on DVE into PLANAR fp8 weight tiles (the dual-fp8 LDWEIGHTS ISA check
  requires planar weights; the moving operand tolerates the interleaved
  layout). fp8 DoubleRow matmuls contract K=256 per instruction at ~2x
  the bf16 MAC rate. The 1/||p|| factors fold into the per-partition
  scale of the Exp activation, which runs in-place on PSUM with a fused
  row-sum accumulator (|cos|<=1 so no max-subtraction needed). The
  diagonal is computed exactly in fp32 from the pred shard and a
  pre-sliced matching target shard (td input keeps the program SPMD).
  Host sums the per-core (lse - diag) partials and divides by N.
"""
import math

import numpy as np

import concourse.bacc as bacc
import concourse.mybir as mybir
import concourse.tile as tile
from concourse.bass_utils import run_bass_kernel_spmd

F32 = mybir.dt.float32
BF16 = mybir.dt.bfloat16
FP8 = mybir.dt.float8e4
U16 = mybir.dt.uint16
AF = mybir.ActivationFunctionType
ALU = mybir.AluOpType
AXIS = mybir.AxisListType
PM = mybir.MatmulPerfMode

TEMPERATURE = 0.5
SCALE = float(math.exp(TEMPERATURE))
FP8_GAIN = 8.0  # normalized target rows scaled by this before fp8 cast

# Full problem config (hardcoded per contest rules).
B, L, C = 4, 2048, 512
N_CORES = 8
N_TOTAL = B * L                  # 8192
M_LOCAL = N_TOTAL // N_CORES     # 1024 rows per core
MT = M_LOCAL // 128              # 8 output row tiles
QROWS = 512                      # rows per staging quad
BLK = 2048                       # target rows per psum block / exp drain
NB = N_TOTAL // BLK              # 4 blocks
NQB = BLK // QROWS               # 4 quads per block
JT = BLK // 512                  # psum 512-slices per block
KQ = C // 256                    # 2 fp8-pair chunks of the contraction


def build_nc():
    """Build + compile the per-core Bass program (SPMD: same NEFF, 8 cores)."""
    nc = bacc.Bacc("TRN2", target_bir_lowering=False, debug=False)
    pred = nc.dram_tensor("pred", [M_LOCAL, C], F32, kind="ExternalInput").ap()
    tgt = nc.dram_tensor("tgt", [N_TOTAL, C], F32, kind="ExternalInput").ap()
    td = nc.dram_tensor("td", [M_LOCAL, C], F32, kind="ExternalInput").ap()
    out = nc.dram_tensor("out", [128, MT], F32, kind="ExternalOutput").ap()

    def quad_dram_ap(t2d, r0, rows):
        # DRAM rows [r0, r0+rows) viewed as [128 part, rows//128, c]
        return t2d[r0:r0 + rows, :].rearrange("(q p) c -> p q c", p=128)

    with tile.TileContext(nc) as tc:
        with (
            tc.tile_pool(name="dram", bufs=1, space="DRAM") as dram_pool,
            tc.tile_pool(name="ld", bufs=1) as ld_pool,
            tc.tile_pool(name="tq", bufs=1) as tq_pool,
            tc.tile_pool(name="q8", bufs=1) as q8_pool,
            tc.tile_pool(name="sq", bufs=2) as sq_pool,
            tc.tile_pool(name="st", bufs=1) as stats_pool,
            tc.tile_pool(name="pT", bufs=1) as pT_pool,
            tc.tile_pool(name="tT", bufs=1) as tT_pool,
            tc.tile_pool(name="ps", bufs=2, space="PSUM") as psum_pool,
        ):
            # ---------------- Phase A: pred shard + exact diagonal ----------
            pq, tdq = [], []
            for k in range(M_LOCAL // QROWS):
                p_ld = ld_pool.tile([128, 4 * C], F32, name=f"pld{k}",
                                    tag=f"pld{k}")
                nc.sync.dma_start(p_ld[:].rearrange("p (q c) -> p q c", c=C),
                                  quad_dram_ap(pred, k * QROWS, QROWS))
                t_ld = ld_pool.tile([128, 4 * C], F32, name=f"tdld{k}",
                                    tag=f"tdld{k}")
                nc.sync.dma_start(t_ld[:].rearrange("p (q c) -> p q c", c=C),
                                  quad_dram_ap(td, k * QROWS, QROWS))
                pq.append(p_ld)
                tdq.append(t_ld)

            sp = stats_pool.tile([128, MT], F32, name="sp", tag="sp")
            std = stats_pool.tile([128, MT], F32, name="std", tag="std")
            d0 = stats_pool.tile([128, MT], F32, name="d0", tag="d0")
            for k in range(M_LOCAL // QROWS):
                for s in range(4):
                    col = k * 4 + s
                    a = pq[k][:, s * C:(s + 1) * C]
                    b = tdq[k][:, s * C:(s + 1) * C]
                    # pred squares on the scalar engine (frees DVE)
                    sqa = sq_pool.tile([128, C], F32, name="sqa", tag="sqa")
                    nc.scalar.activation(sqa[:], a, AF.Square,
                                         accum_out=sp[:, col:col + 1])
                    sqb = sq_pool.tile([128, C], F32, name="sqb", tag="sqb")
                    nc.vector.scalar_tensor_tensor(
                        sqb[:], b, 1.0, b, ALU.mult, ALU.mult,
                        accum_out=std[:, col:col + 1])
                    sqc = sq_pool.tile([128, C], F32, name="sqc", tag="sqb")
                    nc.vector.scalar_tensor_tensor(
                        sqc[:], a, 1.0, b, ALU.mult, ALU.mult,
                        accum_out=d0[:, col:col + 1])

            lp = stats_pool.tile([128, MT], F32, name="lp", tag="lp")
            lt = stats_pool.tile([128, MT], F32, name="lt", tag="lt")
            nc.scalar.activation(lp[:], sp[:], AF.Ln)
            nc.scalar.activation(lt[:], std[:], AF.Ln)
            rp = stats_pool.tile([128, MT], F32, name="rp", tag="rp")
            rtd = stats_pool.tile([128, MT], F32, name="rtd", tag="rtd")
            nc.scalar.activation(rp[:], lp[:], AF.Exp, scale=-0.5)
            nc.scalar.activation(rtd[:], lt[:], AF.Exp, scale=-0.5)

            expsc = stats_pool.tile([128, MT], F32, name="expsc", tag="expsc")
            nc.vector.tensor_scalar_mul(expsc[:], rp[:], SCALE / FP8_GAIN)
            dtmp = stats_pool.tile([128, MT], F32, name="dtmp", tag="dtmp")
            nc.vector.tensor_mul(dtmp[:], d0[:], rtd[:])
            diag = stats_pool.tile([128, MT], F32, name="diag", tag="diag")
            nc.vector.scalar_tensor_tensor(
                diag[:], dtmp[:], SCALE, rp[:], ALU.mult, ALU.mult)

            # pred: raw bf16 bounce -> exact u16 xbar transpose -> planar fp8
            pb_dram = dram_pool.tile([M_LOCAL, C], BF16, name="pbd", tag="pbd")
            for k in range(M_LOCAL // QROWS):
                pbq = q8_pool.tile([128, 4 * C], BF16, name=f"pbq{k}",
                                   tag=f"pbq{k}")
                for s in range(4):
                    nc.scalar.copy(pbq[:, s * C:(s + 1) * C],
                                   pq[k][:, s * C:(s + 1) * C])
                nc.gpsimd.dma_start(quad_dram_ap(pb_dram, k * QROWS, QROWS),
                                    pbq[:].rearrange("p (q c) -> p q c", c=C))
            pTb = []
            for kc in range(C // 128):
                pt = pT_pool.tile([128, M_LOCAL], BF16, name=f"pTb{kc}",
                                  tag=f"pTb{kc}")
                nc.sync.dma_start_transpose(
                    pt[:], pb_dram[0:M_LOCAL, kc * 128:(kc + 1) * 128])
                pTb.append(pt)
            pw = []
            for q in range(KQ):
                w = pT_pool.tile([128, 2 * M_LOCAL], FP8, name=f"pw{q}",
                                 tag=f"pw{q}")
                for i in range(2):
                    nc.vector.tensor_scalar_mul(
                        w[:, i * M_LOCAL:(i + 1) * M_LOCAL],
                        pTb[2 * q + i][:], 1.0)
                pw.append(w)

            # ---------------- Phase B: target blocks ------------------------
            # Per block: load fp32 quads, row norms, permuted fp8 cast,
            # bounce to DRAM, u16 xbar transpose, DoubleRow matmuls, exp.
            t8_dram = dram_pool.tile([N_TOTAL, C], FP8, name="t8d", tag="t8d")
            t8u = t8_dram.bitcast(U16)  # [N_TOTAL, 256] u16
            sume = stats_pool.tile([128, MT * NB], F32, name="sume",
                                   tag="sume")

            tq_tiles = {}
            # prefetch loads of the first two blocks (sync queue order)
            for g in range(min(2, NB)):
                for qi in range(NQB):
                    t_ld = tq_pool.tile([128, 4 * C], F32, name="tqd",
                                        tag="tqd", bufs=3 * NQB)
                    nc.sync.dma_start(
                        t_ld[:].rearrange("p (q c) -> p q c", c=C),
                        quad_dram_ap(tgt, g * BLK + qi * QROWS, QROWS))
                    tq_tiles[(g, qi)] = t_ld

            for g in range(NB):
                # norms for this block
                stt = stats_pool.tile([128, BLK // 128], F32, name="stt",
                                      tag="stt", bufs=2)
                for qi in range(NQB):
                    tqd = tq_tiles[(g, qi)]
                    for s in range(4):
                        a = tqd[:, s * C:(s + 1) * C]
                        sqd = sq_pool.tile([128, C], F32, name="sqd",
                                           tag="sqd")
                        nc.vector.scalar_tensor_tensor(
                            sqd[:], a, 1.0, a, ALU.mult, ALU.mult,
                            accum_out=stt[:, qi * 4 + s:qi * 4 + s + 1])
                ltt = stats_pool.tile([128, BLK // 128], F32, name="ltt",
                                      tag="ltt", bufs=2)
                rtt = stats_pool.tile([128, BLK // 128], F32, name="rtt",
                                      tag="rtt", bufs=2)
                nc.scalar.activation(ltt[:], stt[:], AF.Ln)
                nc.scalar.activation(rtt[:], ltt[:], AF.Exp, scale=-0.5)

                # permuted fp8 cast + bounce (gpsimd queue)
                for qi in range(NQB):
                    tqd = tq_tiles[(g, qi)]
                    t8q = q8_pool.tile([128, 4 * C], FP8, name="t8q",
                                       tag="t8q", bufs=2 * NQB)
                    for s in range(4):
                        col = qi * 4 + s
                        t8_out = t8q[:, s * C:(s + 1) * C].rearrange(
                            "p (q j i) -> p q i j", q=2, j=128, i=2)
                        t8_in = tqd[:, s * C:(s + 1) * C].rearrange(
                            "p (q i j) -> p q i j", q=2, i=2, j=128)
                        nc.vector.tensor_scalar(
                            t8_out, t8_in, rtt[:, col:col + 1], FP8_GAIN,
                            ALU.mult, ALU.mult)
                    nc.gpsimd.dma_start(
                        quad_dram_ap(t8_dram, g * BLK + qi * QROWS, QROWS),
                        t8q[:].rearrange("p (q c) -> p q c", c=C))

                # prefetch loads for block g+2 (before this block's
                # transposes so the sync queue never stalls the loads)
                gp = g + 2
                if gp < NB:
                    for qi in range(NQB):
                        t_ld = tq_pool.tile([128, 4 * C], F32, name="tqd",
                                            tag="tqd", bufs=3 * NQB)
                        nc.sync.dma_start(
                            t_ld[:].rearrange("p (q c) -> p q c", c=C),
                            quad_dram_ap(tgt, gp * BLK + qi * QROWS, QROWS))
                        tq_tiles[(gp, qi)] = t_ld

                # u16-packed xbar transposes of this block
                tTg = []
                for q in range(KQ):
                    tt = tT_pool.tile([128, BLK], U16, name="tT", tag="tT",
                                      bufs=4)
                    nc.sync.dma_start_transpose(
                        tt[:],
                        t8u[g * BLK:(g + 1) * BLK, q * 128:(q + 1) * 128])
                    tTg.append(tt)

                # DoubleRow matmuls + in-place exp with fused row-sums
                for m in range(MT):
                    ps = psum_pool.tile([128, BLK], F32, name="ps", tag="ps")
                    for q in range(KQ):
                        w_ap = pw[q].rearrange(
                            "j (i m) -> j i m",
                            i=2)[:, :, 128 * m:128 * (m + 1)]
                        for j in range(JT):
                            x_ap = tTg[q].bitcast(FP8)[
                                :, 1024 * j:1024 * (j + 1)].rearrange(
                                    "j (n i) -> j i n", i=2)
                            nc.tensor.matmul(
                                ps[:, j * 512:(j + 1) * 512], w_ap, x_ap,
                                start=(q == 0), stop=(q == KQ - 1),
                                perf_mode=PM.DoubleRow)
                    nc.scalar.activation(
                        ps[:], ps[:], AF.Exp, scale=expsc[:, m:m + 1],
                        accum_out=sume[:, m * NB + g:m * NB + g + 1])

            # ---------------- Phase C: lse - diag --------------------------
            rowsum = stats_pool.tile([128, MT], F32, name="rowsum",
                                     tag="rowsum")
            nc.vector.tensor_reduce(
                rowsum[:], sume[:].rearrange("p (m g) -> p m g", g=NB),
                axis=AXIS.X, op=ALU.add)
            lse = stats_pool.tile([128, MT], F32, name="lse", tag="lse")
            nc.scalar.activation(lse[:], rowsum[:], AF.Ln)
            losst = stats_pool.tile([128, MT], F32, name="losst", tag="losst")
            nc.vector.tensor_sub(losst[:], lse[:], diag[:])
            nc.sync.dma_start(out[:], losst[:])

    nc.compile()
    return nc


_NC_CACHE = {}


def _get_nc():
    key = (M_LOCAL, N_TOTAL, C)
    if key not in _NC_CACHE:
        _NC_CACHE[key] = build_nc()
    return _NC_CACHE[key]


def run_cores(pred2d, tgt2d, trace=False):
    """Run the SPMD program on cores 0..7; returns (partials [8,128,MT], res)."""
    nc = _get_nc()
    in_maps = []
    for ci in range(N_CORES):
        r0 = ci * M_LOCAL
        in_maps.append({
            "pred": np.ascontiguousarray(pred2d[r0:r0 + M_LOCAL]),
            "tgt": np.ascontiguousarray(tgt2d),
            "td": np.ascontiguousarray(tgt2d[r0:r0 + M_LOCAL]),
        })
    res = run_bass_kernel_spmd(nc, in_maps, list(range(N_CORES)), trace=trace)
    partials = np.stack([res.results[i]["out"] for i in range(N_CORES)])
    return partials, res


def kernel(pred, target):
    pred2d = np.asarray(pred, dtype=np.float32).reshape(-1, C)
    tgt2d = np.asarray(target, dtype=np.float32).reshape(-1, C)
    partials, _ = run_cores(pred2d, tgt2d)
    loss = partials.astype(np.float64).sum() / float(N_TOTAL)
    return np.float32(loss)


# revision 10
# speedup vs baseline: 1.3669x; 1.0372x over previous
"""Trainium2 Bass kernel for cosine-similarity contrastive loss (CosSimLoss).

reference:
    p = l2norm(pred).reshape(-1, C); t = l2norm(target).reshape(-1, C)
    logits = (p @ t.T) * e^0.5
    loss = mean(logsumexp(logits, axis=1) - diag(logits))

Strategy (8 NeuronCores, data parallel over N = B*L = 8192 rows of pred):
  Each core gets a 1024-row shard of pred plus the full target. Per
  2048-row target block it computes row norms (DVE square-accum), scales
  by 8/||t|| and casts to fp8e4 with a column-PERMUTED write so that the
  uint16-packed DMA xbar transpose lands the moving operand with the
  byte-interleaved (j, i) -> c map c = 256q + 128i + j. pred stays RAW:
  it is cast to bf16, transposed exactly via the 2-byte xbar, then cast
  on DVE into PLANAR fp8 weight tiles (the dual-fp8 LDWEIGHTS ISA check
  requires planar weights; the moving operand tolerates the interleaved
  layout). fp8 DoubleRow matmuls contract K=256 per instruction at ~2x
  the bf16 MAC rate. The 1/||p|| factors fold into the per-partition
  scale of the Exp activation, which runs in-place on PSUM with a fused
  row-sum accumulator (|cos|<=1 so no max-subtraction needed). The
  diagonal is computed exactly in fp32 from the pred shard and a
  pre-sliced matching target shard (td input keeps the program SPMD).
  Host sums the per-core (lse - diag) partials and divides by N.
"""
import math

import numpy as np

import concourse.bacc as bacc
import concourse.mybir as mybir
import concourse.tile as tile
from concourse.bass_utils import run_bass_kernel_spmd

F32 = mybir.dt.float32
BF16 = mybir.dt.bfloat16
FP8 = mybir.dt.float8e4
U16 = mybir.dt.uint16
AF = mybir.ActivationFunctionType
ALU = mybir.AluOpType
AXIS = mybir.AxisListType
PM = mybir.MatmulPerfMode

TEMPERATURE = 0.5
SCALE = float(math.exp(TEMPERATURE))
FP8_GAIN = 8.0  # normalized target rows scaled by this before fp8 cast

# Full problem config (hardcoded per contest rules).
B, L, C = 4, 2048, 512
N_CORES = 8
N_TOTAL = B * L                  # 8192
M_LOCAL = N_TOTAL // N_CORES     # 1024 rows per core
MT = M_LOCAL // 128              # 8 output row tiles
QROWS = 512                      # rows per staging quad
BLK = 2048                       # target rows per psum block / exp drain
NB = N_TOTAL // BLK              # 4 blocks
NQB = BLK // QROWS               # 4 quads per block
JT = BLK // 512                  # psum 512-slices per block
KQ = C // 256                    # 2 fp8-pair chunks of the contraction


def build_nc():
    """Build + compile the per-core Bass program (SPMD: same NEFF, 8 cores)."""
    nc = bacc.Bacc("TRN2", target_bir_lowering=False, debug=False)
    pred = nc.dram_tensor("pred", [M_LOCAL, C], F32, kind="ExternalInput").ap()
    tgt = nc.dram_tensor("tgt", [N_TOTAL, C], F32, kind="ExternalInput").ap()
    td = nc.dram_tensor("td", [M_LOCAL, C], F32, kind="ExternalInput").ap()
    out = nc.dram_tensor("out", [128, MT], F32, kind="ExternalOutput").ap()

    def quad_dram_ap(t2d, r0, rows):
        # DRAM rows [r0, r0+rows) viewed as [128 part, rows//128, c]
        return t2d[r0:r0 + rows, :].rearrange("(q p) c -> p q c", p=128)

    with tile.TileContext(nc) as tc:
        with (
            tc.tile_pool(name="dram", bufs=1, space="DRAM") as dram_pool,
            tc.tile_pool(name="ld", bufs=1) as ld_pool,
            tc.tile_pool(name="tq", bufs=1) as tq_pool,
            tc.tile_pool(name="q8", bufs=1) as q8_pool,
            tc.tile_pool(name="sq", bufs=2) as sq_pool,
            tc.tile_pool(name="st", bufs=1) as stats_pool,
            tc.tile_pool(name="pT", bufs=1) as pT_pool,
            tc.tile_pool(name="tT", bufs=1) as tT_pool,
            tc.tile_pool(name="ps", bufs=2, space="PSUM") as psum_pool,
        ):
            # ---------------- Phase A: pred shard + exact diagonal ----------
            pq, tdq = [], []
            for k in range(M_LOCAL // QROWS):
                p_ld = ld_pool.tile([128, 4 * C], F32, name=f"pld{k}",
                                    tag=f"pld{k}")
                nc.sync.dma_start(p_ld[:].rearrange("p (q c) -> p q c", c=C),
                                  quad_dram_ap(pred, k * QROWS, QROWS))
                t_ld = ld_pool.tile([128, 4 * C], F32, name=f"tdld{k}",
                                    tag=f"tdld{k}")
                nc.sync.dma_start(t_ld[:].rearrange("p (q c) -> p q c", c=C),
                                  quad_dram_ap(td, k * QROWS, QROWS))
                pq.append(p_ld)
                tdq.append(t_ld)

            sp = stats_pool.tile([128, MT], F32, name="sp", tag="sp")
            std = stats_pool.tile([128, MT], F32, name="std", tag="std")
            d0 = stats_pool.tile([128, MT], F32, name="d0", tag="d0")
            for k in range(M_LOCAL // QROWS):
                for s in range(4):
                    col = k * 4 + s
                    a = pq[k][:, s * C:(s + 1) * C]
                    b = tdq[k][:, s * C:(s + 1) * C]
                    # pred squares on the scalar engine (frees DVE)
                    sqa = sq_pool.tile([128, C], F32, name="sqa", tag="sqa")
                    nc.scalar.activation(sqa[:], a, AF.Square,
                                         accum_out=sp[:, col:col + 1])
                    sqb = sq_pool.tile([128, C], F32, name="sqb", tag="sqb")
                    nc.vector.scalar_tensor_tensor(
                        sqb[:], b, 1.0, b, ALU.mult, ALU.mult,
                        accum_out=std[:, col:col + 1])
                    sqc = sq_pool.tile([128, C], F32, name="sqc", tag="sqb")
                    nc.vector.scalar_tensor_tensor(
                        sqc[:], a, 1.0, b, ALU.mult, ALU.mult,
                        accum_out=d0[:, col:col + 1])

            # pred: raw bf16 bounce -> exact u16 xbar transpose -> planar fp8
            pb_dram = dram_pool.tile([M_LOCAL, C], BF16, name="pbd", tag="pbd")
            for k in range(M_LOCAL // QROWS):
                pbq = q8_pool.tile([128, 4 * C], BF16, name=f"pbq{k}",
                                   tag=f"pbq{k}")
                for s in range(4):
                    nc.scalar.copy(pbq[:, s * C:(s + 1) * C],
                                   pq[k][:, s * C:(s + 1) * C])
                nc.gpsimd.dma_start(quad_dram_ap(pb_dram, k * QROWS, QROWS),
                                    pbq[:].rearrange("p (q c) -> p q c", c=C))

            # ---------------- Phase B: target blocks ------------------------
            # Per block: load fp32 quads, row norms, permuted fp8 cast,
            # bounce to DRAM, u16 xbar transpose, DoubleRow matmuls, exp.
            t8_dram = dram_pool.tile([N_TOTAL, C], FP8, name="t8d", tag="t8d")
            t8u = t8_dram.bitcast(U16)  # [N_TOTAL, 256] u16
            sume = stats_pool.tile([128, MT * NB], F32, name="sume",
                                   tag="sume")

            tq_tiles = {}

            def load_block(g):
                for qi in range(NQB):
                    t_ld = tq_pool.tile([128, 4 * C], F32, name="tqd",
                                        tag="tqd", bufs=3 * NQB)
                    nc.sync.dma_start(
                        t_ld[:].rearrange("p (q c) -> p q c", c=C),
                        quad_dram_ap(tgt, g * BLK + qi * QROWS, QROWS))
                    tq_tiles[(g, qi)] = t_ld

            def block_norms(g):
                # DVE square-accum for the block's row norms
                stt = stats_pool.tile([128, BLK // 128], F32, name="stt",
                                      tag="stt", bufs=2)
                for qi in range(NQB):
                    tqd = tq_tiles[(g, qi)]
                    for s in range(4):
                        a = tqd[:, s * C:(s + 1) * C]
                        sqd = sq_pool.tile([128, C], F32, name="sqd",
                                           tag="sqd")
                        nc.vector.scalar_tensor_tensor(
                            sqd[:], a, 1.0, a, ALU.mult, ALU.mult,
                            accum_out=stt[:, qi * 4 + s:qi * 4 + s + 1])
                return stt

            def block_rsqrt(stts):
                # Ln then Exp grouped across blocks to minimize act-table
                # swaps on the scalar engine.
                outs = []
                ltts = []
                for g, stt in stts:
                    ltt = stats_pool.tile([128, BLK // 128], F32, name="ltt",
                                          tag="ltt", bufs=2)
                    nc.scalar.activation(ltt[:], stt[:], AF.Ln)
                    ltts.append(ltt)
                for (g, stt), ltt in zip(stts, ltts):
                    rtt = stats_pool.tile([128, BLK // 128], F32, name="rtt",
                                          tag="rtt", bufs=4)
                    nc.scalar.activation(rtt[:], ltt[:], AF.Exp, scale=-0.5)
                    outs.append(rtt)
                return outs

            def block_cast_bounce(g, rtt):
                # permuted fp8 cast (DVE) + one merged bounce DMA (gpsimd)
                t8b = q8_pool.tile([128, NQB * 4 * C], FP8, name="t8b",
                                   tag="t8b", bufs=2)
                for qi in range(NQB):
                    tqd = tq_tiles[(g, qi)]
                    for s in range(4):
                        col = qi * 4 + s
                        t8_out = t8b[:, col * C:(col + 1) * C].rearrange(
                            "p (q j i) -> p q i j", q=2, j=128, i=2)
                        t8_in = tqd[:, s * C:(s + 1) * C].rearrange(
                            "p (q i j) -> p q i j", q=2, i=2, j=128)
                        nc.vector.tensor_scalar(
                            t8_out, t8_in, rtt[:, col:col + 1], FP8_GAIN,
                            ALU.mult, ALU.mult)
                nc.gpsimd.dma_start(
                    quad_dram_ap(t8_dram, g * BLK, BLK),
                    t8b[:].rearrange("p (q c) -> p q c", c=C))

            def block_transpose(g):
                tTg = []
                for q in range(KQ):
                    tt = tT_pool.tile([128, BLK], U16, name="tT", tag="tT",
                                      bufs=4)
                    nc.sync.dma_start_transpose(
                        tt[:],
                        t8u[g * BLK:(g + 1) * BLK, q * 128:(q + 1) * 128])
                    tTg.append(tt)
                return tTg

            def block_matmul(g, tTg):
                for m in range(MT):
                    ps = psum_pool.tile([128, BLK], F32, name="ps", tag="ps")
                    for q in range(KQ):
                        w_ap = pw[q].rearrange(
                            "j (i m) -> j i m",
                            i=2)[:, :, 128 * m:128 * (m + 1)]
                        for j in range(JT):
                            x_ap = tTg[q].bitcast(FP8)[
                                :, 1024 * j:1024 * (j + 1)].rearrange(
                                    "j (n i) -> j i n", i=2)
                            nc.tensor.matmul(
                                ps[:, j * 512:(j + 1) * 512], w_ap, x_ap,
                                start=(q == 0), stop=(q == KQ - 1),
                                perf_mode=PM.DoubleRow)
                    nc.scalar.activation(
                        ps[:], ps[:], AF.Exp, scale=expsc[:, m:m + 1],
                        accum_out=sume[:, m * NB + g:m * NB + g + 1])

            # prefetch the first three blocks' loads up front
            for g in range(min(3, NB)):
                load_block(g)

            # block 0 norms, then pred rsqrt + block-0 rsqrt grouped
            stt0 = block_norms(0)
            lp = stats_pool.tile([128, MT], F32, name="lp", tag="lp")
            lt = stats_pool.tile([128, MT], F32, name="lt", tag="lt")
            nc.scalar.activation(lp[:], sp[:], AF.Ln)
            nc.scalar.activation(lt[:], std[:], AF.Ln)
            ltt0 = stats_pool.tile([128, BLK // 128], F32, name="ltt",
                                   tag="ltt", bufs=2)
            nc.scalar.activation(ltt0[:], stt0[:], AF.Ln)
            rp = stats_pool.tile([128, MT], F32, name="rp", tag="rp")
            rtd = stats_pool.tile([128, MT], F32, name="rtd", tag="rtd")
            nc.scalar.activation(rp[:], lp[:], AF.Exp, scale=-0.5)
            nc.scalar.activation(rtd[:], lt[:], AF.Exp, scale=-0.5)
            rtt0 = stats_pool.tile([128, BLK // 128], F32, name="rtt",
                                   tag="rtt", bufs=4)
            nc.scalar.activation(rtt0[:], ltt0[:], AF.Exp, scale=-0.5)

            expsc = stats_pool.tile([128, MT], F32, name="expsc", tag="expsc")
            nc.vector.tensor_scalar_mul(expsc[:], rp[:], SCALE / FP8_GAIN)
            dtmp = stats_pool.tile([128, MT], F32, name="dtmp", tag="dtmp")
            nc.vector.tensor_mul(dtmp[:], d0[:], rtd[:])
            diag = stats_pool.tile([128, MT], F32, name="diag", tag="diag")
            nc.vector.scalar_tensor_tensor(
                diag[:], dtmp[:], SCALE, rp[:], ALU.mult, ALU.mult)

            # block 0 cast/bounce; pred transposes + planar weights next so
            # they don't block the DVE/scalar queues ahead of block 0
            block_cast_bounce(0, rtt0)
            pTb = []
            for kc in range(C // 128):
                pt = pT_pool.tile([128, M_LOCAL], BF16, name=f"pTb{kc}",
                                  tag=f"pTb{kc}")
                nc.sync.dma_start_transpose(
                    pt[:], pb_dram[0:M_LOCAL, kc * 128:(kc + 1) * 128])
                pTb.append(pt)
            pw = []
            for q in range(KQ):
                w = pT_pool.tile([128, 2 * M_LOCAL], FP8, name=f"pw{q}",
                                 tag=f"pw{q}")
                for i in range(2):
                    nc.vector.tensor_scalar_mul(
                        w[:, i * M_LOCAL:(i + 1) * M_LOCAL],
                        pTb[2 * q + i][:], 1.0)
                pw.append(w)

            # blocks 1+2: norms together, rsqrt grouped (one table swap pair)
            stt1 = block_norms(1)
            stt2 = block_norms(2)
            rtt1, rtt2 = block_rsqrt([(1, stt1), (2, stt2)])
            block_cast_bounce(1, rtt1)
            load_block(3)
            tT0 = block_transpose(0)
            block_matmul(0, tT0)
            block_cast_bounce(2, rtt2)
            stt3 = block_norms(3)
            (rtt3,) = block_rsqrt([(3, stt3)])
            tT1 = block_transpose(1)
            block_matmul(1, tT1)
            block_cast_bounce(3, rtt3)
            tT2 = block_transpose(2)
            block_matmul(2, tT2)
            tT3 = block_transpose(3)
            block_matmul(3, tT3)

            # ---------------- Phase C: lse - diag --------------------------
            rowsum = stats_pool.tile([128, MT], F32, name="rowsum",
                                     tag="rowsum")
            nc.vector.tensor_reduce(
                rowsum[:], sume[:].rearrange("p (m g) -> p m g", g=NB),
                axis=AXIS.X, op=ALU.add)
            lse = stats_pool.tile([128, MT], F32, name="lse", tag="lse")
            nc.scalar.activation(lse[:], rowsum[:], AF.Ln)
            losst = stats_pool.tile([128, MT], F32, name="losst", tag="losst")
            nc.vector.tensor_sub(losst[:], lse[:], diag[:])
            nc.sync.dma_start(out[:], losst[:])

    nc.compile()
    return nc


_NC_CACHE = {}


def _get_nc():
    key = (M_LOCAL, N_TOTAL, C)
    if key not in _NC_CACHE:
        _NC_CACHE[key] = build_nc()
    return _NC_CACHE[key]


def run_cores(pred2d, tgt2d, trace=False):
    """Run the SPMD program on cores 0..7; returns (partials [8,128,MT], res)."""
    nc = _get_nc()
    in_maps = []
    for ci in range(N_CORES):
        r0 = ci * M_LOCAL
        in_maps.append({
            "pred": np.ascontiguousarray(pred2d[r0:r0 + M_LOCAL]),
            "tgt": np.ascontiguousarray(tgt2d),
            "td": np.ascontiguousarray(tgt2d[r0:r0 + M_LOCAL]),
        })
    res = run_bass_kernel_spmd(nc, in_maps, list(range(N_CORES)), trace=trace)
    partials = np.stack([res.results[i]["out"] for i in range(N_CORES)])
    return partials, res


def kernel(pred, target):
    pred2d = np.asarray(pred, dtype=np.float32).reshape(-1, C)
    tgt2d = np.asarray(target, dtype=np.float32).reshape(-1, C)
    partials, _ = run_cores(pred2d, tgt2d)
    loss = partials.astype(np.float64).sum() / float(N_TOTAL)
    return np.float32(loss)


# revision 13
# speedup vs baseline: 1.5041x; 1.1003x over previous
"""Trainium2 Bass kernel for cosine-similarity contrastive loss (CosSimLoss).

reference:
    p = l2norm(pred).reshape(-1, C); t = l2norm(target).reshape(-1, C)
    logits = (p @ t.T) * e^0.5
    loss = mean(logsumexp(logits, axis=1) - diag(logits))

Strategy (8 NeuronCores, data parallel over N = B*L = 8192 rows of pred):
  Each core gets a 1024-row shard of pred plus the full target. Per
  2048-row target block it computes row norms (DVE square-accum), scales
  by 8/||t|| and casts to fp8e4 with a column-PERMUTED write so that the
  uint16-packed DMA xbar transpose lands the moving operand with the
  byte-interleaved (j, i) -> c map c = 256q + 128i + j. pred stays RAW:
  it is cast to bf16, transposed exactly via the 2-byte xbar, then cast
  on DVE into PLANAR fp8 weight tiles (the dual-fp8 LDWEIGHTS ISA check
  requires planar weights; the moving operand tolerates the interleaved
  layout). fp8 DoubleRow matmuls contract K=256 per instruction at ~2x
  the bf16 MAC rate. The 1/||p|| factors fold into the per-partition
  scale of the Exp activation, which runs in-place on PSUM with a fused
  row-sum accumulator (|cos|<=1 so no max-subtraction needed). The
  diagonal is computed exactly in fp32 from the pred shard and a
  pre-sliced matching target shard (td input keeps the program SPMD).
  Host sums the per-core (lse - diag) partials and divides by N.
"""
import math

import numpy as np

import concourse.bacc as bacc
import concourse.mybir as mybir
import concourse.tile as tile
from concourse.bass_utils import run_bass_kernel_spmd

F32 = mybir.dt.float32
BF16 = mybir.dt.bfloat16
FP8 = mybir.dt.float8e4
U16 = mybir.dt.uint16
AF = mybir.ActivationFunctionType
ALU = mybir.AluOpType
AXIS = mybir.AxisListType
PM = mybir.MatmulPerfMode

TEMPERATURE = 0.5
SCALE = float(math.exp(TEMPERATURE))
FP8_GAIN = 8.0  # normalized target rows scaled by this before fp8 cast

# Full problem config (hardcoded per contest rules).
B, L, C = 4, 2048, 512
N_CORES = 8
N_TOTAL = B * L                  # 8192
M_LOCAL = N_TOTAL // N_CORES     # 1024 rows per core
MT = M_LOCAL // 128              # 8 output row tiles
QROWS = 512                      # rows per staging quad
BLK = 2048                       # target rows per psum block / exp drain
NB = N_TOTAL // BLK              # 4 blocks
NQB = BLK // QROWS               # 4 quads per block
JT = BLK // 512                  # psum 512-slices per block
KQ = C // 256                    # 2 fp8-pair chunks of the contraction


def build_nc():
    """Build + compile the per-core Bass program (SPMD: same NEFF, 8 cores)."""
    nc = bacc.Bacc("TRN2", target_bir_lowering=False, debug=False)
    pred = nc.dram_tensor("pred", [M_LOCAL, C], F32, kind="ExternalInput").ap()
    tgt = nc.dram_tensor("tgt", [N_TOTAL, C], F32, kind="ExternalInput").ap()
    td = nc.dram_tensor("td", [M_LOCAL, C], F32, kind="ExternalInput").ap()
    out = nc.dram_tensor("out", [128, MT], F32, kind="ExternalOutput").ap()

    def quad_dram_ap(t2d, r0, rows):
        # DRAM rows [r0, r0+rows) viewed as [128 part, rows//128, c]
        return t2d[r0:r0 + rows, :].rearrange("(q p) c -> p q c", p=128)

    with tile.TileContext(nc) as tc:
        with (
            tc.tile_pool(name="dram", bufs=1, space="DRAM") as dram_pool,
            tc.tile_pool(name="ld", bufs=1) as ld_pool,
            tc.tile_pool(name="tq", bufs=1) as tq_pool,
            tc.tile_pool(name="q8", bufs=1) as q8_pool,
            tc.tile_pool(name="sq", bufs=2) as sq_pool,
            tc.tile_pool(name="st", bufs=1) as stats_pool,
            tc.tile_pool(name="pT", bufs=1) as pT_pool,
            tc.tile_pool(name="tT", bufs=1) as tT_pool,
            tc.tile_pool(name="ps", bufs=2, space="PSUM") as psum_pool,
        ):
            # Ramped block sizes: tiny first blocks collapse the startup
            # latency (first matmul ~20us instead of ~85us).
            BLOCKS = [(0, 512), (512, 512), (1024, 1024),
                      (2048, 2048), (4096, 2048), (6144, 2048)]
            NBLK = len(BLOCKS)

            t8_dram = dram_pool.tile([N_TOTAL, C], FP8, name="t8d", tag="t8d")
            t8u = t8_dram.bitcast(U16)  # [N_TOTAL, 256] u16
            sume = stats_pool.tile([128, MT * NBLK], F32, name="sume",
                                   tag="sume")
            pb_dram = dram_pool.tile([M_LOCAL, C], BF16, name="pbd", tag="pbd")

            tq_tiles = {}

            def load_block(g):
                goff, bsz = BLOCKS[g]
                for qi in range(bsz // QROWS):
                    t_ld = tq_pool.tile([128, 4 * C], F32, name="tqd",
                                        tag="tqd", bufs=12)
                    nc.sync.dma_start(
                        t_ld[:].rearrange("p (q c) -> p q c", c=C),
                        quad_dram_ap(tgt, goff + qi * QROWS, QROWS))
                    tq_tiles[(g, qi)] = t_ld

            def block_norms(g):
                goff, bsz = BLOCKS[g]
                stt = stats_pool.tile([128, bsz // 128], F32, name="stt",
                                      tag=f"stt{g}")
                for qi in range(bsz // QROWS):
                    tqd = tq_tiles[(g, qi)]
                    for s in range(4):
                        a = tqd[:, s * C:(s + 1) * C]
                        sqd = sq_pool.tile([128, C], F32, name="sqd",
                                           tag="sqd")
                        nc.vector.scalar_tensor_tensor(
                            sqd[:], a, 1.0, a, ALU.mult, ALU.mult,
                            accum_out=stt[:, qi * 4 + s:qi * 4 + s + 1])
                return stt

            def rsqrt_group(stts):
                # Ln batch then Exp batch: at most one act-table swap pair
                outs = {}
                ltts = []
                for key, stt, cols in stts:
                    ltt = stats_pool.tile([128, cols], F32, name="ltt",
                                          tag=f"ltt{key}")
                    nc.scalar.activation(ltt[:], stt[:], AF.Ln)
                    ltts.append(ltt)
                for (key, stt, cols), ltt in zip(stts, ltts):
                    rtt = stats_pool.tile([128, cols], F32, name="rtt",
                                          tag=f"rtt{key}")
                    nc.scalar.activation(rtt[:], ltt[:], AF.Exp, scale=-0.5)
                    outs[key] = rtt
                return outs

            def block_cast_bounce(g, rtt):
                # permuted fp8 cast (DVE) + one merged bounce DMA (gpsimd)
                goff, bsz = BLOCKS[g]
                t8b = q8_pool.tile([128, (bsz // 128) * C], FP8, name="t8b",
                                   tag="t8b", bufs=3, padded_shape=[128, 16 * C])
                for qi in range(bsz // QROWS):
                    tqd = tq_tiles[(g, qi)]
                    for s in range(4):
                        col = qi * 4 + s
                        t8_out = t8b[:, col * C:(col + 1) * C].rearrange(
                            "p (q j i) -> p q i j", q=2, j=128, i=2)
                        t8_in = tqd[:, s * C:(s + 1) * C].rearrange(
                            "p (q i j) -> p q i j", q=2, i=2, j=128)
                        nc.vector.tensor_scalar(
                            t8_out, t8_in, rtt[:, col:col + 1], FP8_GAIN,
                            ALU.mult, ALU.mult)
                nc.gpsimd.dma_start(
                    quad_dram_ap(t8_dram, goff, bsz),
                    t8b[:, :(bsz // 128) * C].rearrange(
                        "p (q c) -> p q c", c=C))

            def block_transpose(g):
                goff, bsz = BLOCKS[g]
                tTg = []
                for q in range(KQ):
                    tt = tT_pool.tile([128, bsz], U16, name="tT", tag="tT",
                                      bufs=4, padded_shape=[128, BLK])
                    nc.sync.dma_start_transpose(
                        tt[:],
                        t8u[goff:goff + bsz, q * 128:(q + 1) * 128])
                    tTg.append(tt)
                return tTg

            def block_matmul(g, tTg):
                goff, bsz = BLOCKS[g]
                for m in range(MT):
                    ps = psum_pool.tile([128, bsz], F32, name="ps", tag="ps",
                                        padded_shape=[128, BLK])
                    for q in range(KQ):
                        w_ap = pw[q].rearrange(
                            "j (i m) -> j i m",
                            i=2)[:, :, 128 * m:128 * (m + 1)]
                        for j in range(bsz // 512):
                            x_ap = tTg[q].bitcast(FP8)[
                                :, 1024 * j:1024 * (j + 1)].rearrange(
                                    "j (n i) -> j i n", i=2)
                            nc.tensor.matmul(
                                ps[:, j * 512:(j + 1) * 512], w_ap, x_ap,
                                start=(q == 0), stop=(q == KQ - 1),
                                perf_mode=PM.DoubleRow)
                    nc.scalar.activation(
                        ps[:], ps[:], AF.Exp, scale=expsc[:, m:m + 1],
                        accum_out=sume[:, m * NBLK + g:m * NBLK + g + 1])

            # ---------------- Phase A/B interleaved pipeline ----------------
            # loads: block 0 first (critical path), then pred/td, then more
            load_block(0)
            pq, tdq = [], []
            for k in range(M_LOCAL // QROWS):
                p_ld = ld_pool.tile([128, 4 * C], F32, name=f"pld{k}",
                                    tag=f"pld{k}")
                nc.sync.dma_start(p_ld[:].rearrange("p (q c) -> p q c", c=C),
                                  quad_dram_ap(pred, k * QROWS, QROWS))
                t_ld = ld_pool.tile([128, 4 * C], F32, name=f"tdld{k}",
                                    tag=f"tdld{k}")
                nc.sync.dma_start(t_ld[:].rearrange("p (q c) -> p q c", c=C),
                                  quad_dram_ap(td, k * QROWS, QROWS))
                pq.append(p_ld)
                tdq.append(t_ld)
            load_block(1)
            load_block(2)
            load_block(3)

            # block 0 norms + rsqrt (first table pair), cast, bounce
            stt0 = block_norms(0)
            rt0 = rsqrt_group([(0, stt0, 4)])
            block_cast_bounce(0, rt0[0])

            # pred bf16 cast (DVE) + bounce, and pred/td norms + diag
            for k in range(M_LOCAL // QROWS):
                pbq = q8_pool.tile([128, 4 * C], BF16, name=f"pbq{k}",
                                   tag=f"pbq{k}")
                for s in range(4):
                    nc.vector.tensor_scalar_mul(
                        pbq[:, s * C:(s + 1) * C],
                        pq[k][:, s * C:(s + 1) * C], 1.0)
                nc.gpsimd.dma_start(quad_dram_ap(pb_dram, k * QROWS, QROWS),
                                    pbq[:].rearrange("p (q c) -> p q c", c=C))
            sp = stats_pool.tile([128, MT], F32, name="sp", tag="sp")
            std = stats_pool.tile([128, MT], F32, name="std", tag="std")
            d0 = stats_pool.tile([128, MT], F32, name="d0", tag="d0")
            for k in range(M_LOCAL // QROWS):
                for s in range(4):
                    col = k * 4 + s
                    a = pq[k][:, s * C:(s + 1) * C]
                    b = tdq[k][:, s * C:(s + 1) * C]
                    sqa = sq_pool.tile([128, C], F32, name="sqa", tag="sqd")
                    nc.vector.scalar_tensor_tensor(
                        sqa[:], a, 1.0, a, ALU.mult, ALU.mult,
                        accum_out=sp[:, col:col + 1])
                    sqb = sq_pool.tile([128, C], F32, name="sqb", tag="sqd")
                    nc.vector.scalar_tensor_tensor(
                        sqb[:], b, 1.0, b, ALU.mult, ALU.mult,
                        accum_out=std[:, col:col + 1])

            # block 1 norms; then grouped rsqrt for (pred, td, block1)
            stt1 = block_norms(1)
            rg = rsqrt_group([("p", sp, MT), ("td", std, MT), (1, stt1, 4)])
            rp, rtd, rtt1 = rg["p"], rg["td"], rg[1]

            expsc = stats_pool.tile([128, MT], F32, name="expsc", tag="expsc")
            nc.vector.tensor_scalar_mul(expsc[:], rp[:], SCALE / FP8_GAIN)

            # pred transposes -> planar fp8 weights (DVE)
            pTb = []
            for kc in range(C // 128):
                pt = pT_pool.tile([128, M_LOCAL], BF16, name=f"pTb{kc}",
                                  tag=f"pTb{kc}")
                nc.sync.dma_start_transpose(
                    pt[:], pb_dram[0:M_LOCAL, kc * 128:(kc + 1) * 128])
                pTb.append(pt)
            pw = []
            for q in range(KQ):
                w = pT_pool.tile([128, 2 * M_LOCAL], FP8, name=f"pw{q}",
                                 tag=f"pw{q}")
                for i in range(2):
                    nc.vector.tensor_scalar_mul(
                        w[:, i * M_LOCAL:(i + 1) * M_LOCAL],
                        pTb[2 * q + i][:], 1.0)
                pw.append(w)

            block_cast_bounce(1, rtt1)
            tT0 = block_transpose(0)
            block_matmul(0, tT0)

            stt2 = block_norms(2)
            rtt2 = rsqrt_group([(2, stt2, 8)])[2]
            block_cast_bounce(2, rtt2)
            tT1 = block_transpose(1)
            block_matmul(1, tT1)

            # deferred diag dot products (DVE idle window), then diag scale
            for k in range(M_LOCAL // QROWS):
                for s in range(4):
                    col = k * 4 + s
                    a = pq[k][:, s * C:(s + 1) * C]
                    b = tdq[k][:, s * C:(s + 1) * C]
                    sqc = sq_pool.tile([128, C], F32, name="sqc", tag="sqd")
                    nc.vector.scalar_tensor_tensor(
                        sqc[:], a, 1.0, b, ALU.mult, ALU.mult,
                        accum_out=d0[:, col:col + 1])
            dtmp = stats_pool.tile([128, MT], F32, name="dtmp", tag="dtmp")
            nc.vector.tensor_mul(dtmp[:], d0[:], rtd[:])
            diag = stats_pool.tile([128, MT], F32, name="diag", tag="diag")
            nc.vector.scalar_tensor_tensor(
                diag[:], dtmp[:], SCALE, rp[:], ALU.mult, ALU.mult)

            load_block(4)
            stt3 = block_norms(3)
            rtt3 = rsqrt_group([(3, stt3, 16)])[3]
            block_cast_bounce(3, rtt3)
            tT2 = block_transpose(2)
            block_matmul(2, tT2)

            load_block(5)
            stt4 = block_norms(4)
            stt5 = block_norms(5)
            rg45 = rsqrt_group([(4, stt4, 16), (5, stt5, 16)])
            block_cast_bounce(4, rg45[4])
            tT3 = block_transpose(3)
            block_matmul(3, tT3)
            block_cast_bounce(5, rg45[5])
            tT4 = block_transpose(4)
            block_matmul(4, tT4)
            tT5 = block_transpose(5)
            block_matmul(5, tT5)

            # ---------------- Phase C: lse - diag --------------------------
            rowsum = stats_pool.tile([128, MT], F32, name="rowsum",
                                     tag="rowsum")
            nc.vector.tensor_reduce(
                rowsum[:], sume[:].rearrange("p (m g) -> p m g", g=NBLK),
                axis=AXIS.X, op=ALU.add)
            lse = stats_pool.tile([128, MT], F32, name="lse", tag="lse")
            nc.scalar.activation(lse[:], rowsum[:], AF.Ln)
            losst = stats_pool.tile([128, MT], F32, name="losst", tag="losst")
            nc.vector.tensor_sub(losst[:], lse[:], diag[:])
            nc.sync.dma_start(out[:], losst[:])

    nc.compile()
    return nc


_NC_CACHE = {}


def _get_nc():
    key = (M_LOCAL, N_TOTAL, C)
    if key not in _NC_CACHE:
        _NC_CACHE[key] = build_nc()
    return _NC_CACHE[key]


def run_cores(pred2d, tgt2d, trace=False):
    """Run the SPMD program on cores 0..7; returns (partials [8,128,MT], res)."""
    nc = _get_nc()
    in_maps = []
    for ci in range(N_CORES):
        r0 = ci * M_LOCAL
        in_maps.append({
            "pred": np.ascontiguousarray(pred2d[r0:r0 + M_LOCAL]),
            "tgt": np.ascontiguousarray(tgt2d),
            "td": np.ascontiguousarray(tgt2d[r0:r0 + M_LOCAL]),
        })
    res = run_bass_kernel_spmd(nc, in_maps, list(range(N_CORES)), trace=trace)
    partials = np.stack([res.results[i]["out"] for i in range(N_CORES)])
    return partials, res


def kernel(pred, target):
    pred2d = np.asarray(pred, dtype=np.float32).reshape(-1, C)
    tgt2d = np.asarray(target, dtype=np.float32).reshape(-1, C)
    partials, _ = run_cores(pred2d, tgt2d)
    loss = partials.astype(np.float64).sum() / float(N_TOTAL)
    return np.float32(loss)


# revision 15
# speedup vs baseline: 1.5452x; 1.0273x over previous
"""Trainium2 Bass kernel for cosine-similarity contrastive loss (CosSimLoss).

reference:
    p = l2norm(pred).reshape(-1, C); t = l2norm(target).reshape(-1, C)
    logits = (p @ t.T) * e^0.5
    loss = mean(logsumexp(logits, axis=1) - diag(logits))

Strategy (8 NeuronCores, data parallel over N = B*L = 8192 rows of pred):
  Each core gets a 1024-row shard of pred plus the full target. Per
  2048-row target block it computes row norms (DVE square-accum), scales
  by 8/||t|| and casts to fp8e4 with a column-PERMUTED write so that the
  uint16-packed DMA xbar transpose lands the moving operand with the
  byte-interleaved (j, i) -> c map c = 256q + 128i + j. pred stays RAW:
  it is cast to bf16, transposed exactly via the 2-byte xbar, then cast
  on DVE into PLANAR fp8 weight tiles (the dual-fp8 LDWEIGHTS ISA check
  requires planar weights; the moving operand tolerates the interleaved
  layout). fp8 DoubleRow matmuls contract K=256 per instruction at ~2x
  the bf16 MAC rate. The 1/||p|| factors fold into the per-partition
  scale of the Exp activation, which runs in-place on PSUM with a fused
  row-sum accumulator (|cos|<=1 so no max-subtraction needed). The
  diagonal is computed exactly in fp32 from the pred shard and a
  pre-sliced matching target shard (td input keeps the program SPMD).
  Host sums the per-core (lse - diag) partials and divides by N.
"""
import math

import numpy as np

import concourse.bacc as bacc
import concourse.mybir as mybir
import concourse.tile as tile
from concourse.bass_utils import run_bass_kernel_spmd

F32 = mybir.dt.float32
BF16 = mybir.dt.bfloat16
FP8 = mybir.dt.float8e4
U16 = mybir.dt.uint16
AF = mybir.ActivationFunctionType
ALU = mybir.AluOpType
AXIS = mybir.AxisListType
PM = mybir.MatmulPerfMode

TEMPERATURE = 0.5
SCALE = float(math.exp(TEMPERATURE))
FP8_GAIN = 8.0  # normalized target rows scaled by this before fp8 cast

# Full problem config (hardcoded per contest rules).
B, L, C = 4, 2048, 512
N_CORES = 8
N_TOTAL = B * L                  # 8192
M_LOCAL = N_TOTAL // N_CORES     # 1024 rows per core
MT = M_LOCAL // 128              # 8 output row tiles
QROWS = 512                      # rows per staging quad
BLK = 2048                       # target rows per psum block / exp drain
NB = N_TOTAL // BLK              # 4 blocks
NQB = BLK // QROWS               # 4 quads per block
JT = BLK // 512                  # psum 512-slices per block
KQ = C // 256                    # 2 fp8-pair chunks of the contraction


def build_nc():
    """Build + compile the per-core Bass program (SPMD: same NEFF, 8 cores)."""
    nc = bacc.Bacc("TRN2", target_bir_lowering=False, debug=False)
    pred = nc.dram_tensor("pred", [M_LOCAL, C], F32, kind="ExternalInput").ap()
    tgt = nc.dram_tensor("tgt", [N_TOTAL, C], F32, kind="ExternalInput").ap()
    td = nc.dram_tensor("td", [M_LOCAL, C], F32, kind="ExternalInput").ap()
    out = nc.dram_tensor("out", [128, MT], F32, kind="ExternalOutput").ap()

    def quad_dram_ap(t2d, r0, rows):
        # DRAM rows [r0, r0+rows) viewed as [128 part, rows//128, c]
        return t2d[r0:r0 + rows, :].rearrange("(q p) c -> p q c", p=128)

    with tile.TileContext(nc) as tc:
        with (
            tc.tile_pool(name="dram", bufs=1, space="DRAM") as dram_pool,
            tc.tile_pool(name="ld", bufs=1) as ld_pool,
            tc.tile_pool(name="tq", bufs=1) as tq_pool,
            tc.tile_pool(name="q8", bufs=1) as q8_pool,
            tc.tile_pool(name="sq", bufs=2) as sq_pool,
            tc.tile_pool(name="st", bufs=1) as stats_pool,
            tc.tile_pool(name="pT", bufs=1) as pT_pool,
            tc.tile_pool(name="tT", bufs=1) as tT_pool,
            tc.tile_pool(name="ps", bufs=2, space="PSUM") as psum_pool,
        ):
            # Ramped block sizes: tiny first blocks collapse the startup
            # latency (first matmul ~20us instead of ~85us).
            BLOCKS = [(0, 512), (512, 512), (1024, 1024),
                      (2048, 2048), (4096, 2048), (6144, 2048)]
            NBLK = len(BLOCKS)

            # Target fp8 bounce is stored as row-PAIR interleaved bytes:
            # DRAM "np-row" np = n//2 holds byte d = 2c + (n%2). The u16
            # xbar transpose of that layout yields PLANAR transposed fp8
            # (partition j = channel 128*qt + j, free = n contiguous),
            # which is what the dual-fp8 matmul ISA wants on both operands.
            t8_dram = dram_pool.tile([N_TOTAL // 2, 2 * C], FP8, name="t8d",
                                     tag="t8d")
            t8u = t8_dram.bitcast(U16)  # [N/2, 512] u16; col u = channel u
            sume = stats_pool.tile([128, MT * NBLK], F32, name="sume",
                                   tag="sume")
            pb_dram = dram_pool.tile([M_LOCAL, C], BF16, name="pbd", tag="pbd")

            tq_tiles = {}

            def load_block(g):
                # rows 4p..4p+3 land on partition p (8KB contiguous reads)
                goff, bsz = BLOCKS[g]
                for qi in range(bsz // QROWS):
                    t_ld = tq_pool.tile([128, 4 * C], F32, name="tqd",
                                        tag="tqd", bufs=12)
                    r0 = goff + qi * QROWS
                    nc.sync.dma_start(
                        t_ld[:].rearrange("p (s c) -> p s c", c=C),
                        tgt[r0:r0 + QROWS, :].rearrange(
                            "(p s) c -> p s c", s=4))
                    tq_tiles[(g, qi)] = t_ld

            def block_norms(g):
                goff, bsz = BLOCKS[g]
                stt = stats_pool.tile([128, bsz // 128], F32, name="stt",
                                      tag=f"stt{g}")
                for qi in range(bsz // QROWS):
                    tqd = tq_tiles[(g, qi)]
                    for s in range(4):
                        a = tqd[:, s * C:(s + 1) * C]
                        sqd = sq_pool.tile([128, C], F32, name="sqd",
                                           tag="sqd")
                        nc.vector.scalar_tensor_tensor(
                            sqd[:], a, 1.0, a, ALU.mult, ALU.mult,
                            accum_out=stt[:, qi * 4 + s:qi * 4 + s + 1])
                return stt

            def rsqrt_group(stts):
                # Ln batch then Exp batch: at most one act-table swap pair
                outs = {}
                ltts = []
                for key, stt, cols in stts:
                    ltt = stats_pool.tile([128, cols], F32, name="ltt",
                                          tag=f"ltt{key}")
                    nc.scalar.activation(ltt[:], stt[:], AF.Ln)
                    ltts.append(ltt)
                for (key, stt, cols), ltt in zip(stts, ltts):
                    rtt = stats_pool.tile([128, cols], F32, name="rtt",
                                          tag=f"rtt{key}")
                    nc.scalar.activation(rtt[:], ltt[:], AF.Exp, scale=-0.5)
                    outs[key] = rtt
                return outs

            def block_cast_bounce(g, rtt):
                # fp8 cast with row-pair interleaved write (DVE) + one
                # merged contiguous bounce DMA (gpsimd)
                goff, bsz = BLOCKS[g]
                nq = bsz // QROWS
                t8b = q8_pool.tile([128, (bsz // 128) * C], FP8, name="t8b",
                                   tag="t8b", bufs=3,
                                   padded_shape=[128, 16 * C])
                for qi in range(nq):
                    tqd = tq_tiles[(g, qi)]
                    for s in range(4):
                        col = qi * 4 + s
                        seg0 = qi * 2048 + (s // 2) * 1024
                        seg = t8b[:, seg0:seg0 + 1024]
                        t8_out = seg.rearrange(
                            "p (c two) -> p two c",
                            two=2)[:, s % 2:s % 2 + 1, :].rearrange(
                                "p a c -> p (a c)")
                        nc.vector.tensor_scalar(
                            t8_out, tqd[:, s * C:(s + 1) * C],
                            rtt[:, col:col + 1], FP8_GAIN,
                            ALU.mult, ALU.mult)
                nc.gpsimd.dma_start(
                    t8_dram[goff // 2:(goff + bsz) // 2, :].rearrange(
                        "(q p t) d -> p q (t d)", p=128, t=2),
                    t8b[:, :(bsz // 128) * C].rearrange(
                        "p (q x) -> p q x", x=2048))

            def block_transpose(g):
                # planar fp8 transposed tiles: tt[q] holds channel planes
                # (2q, 2q+1); plane i partition j = channel 256q + 128i + j
                goff, bsz = BLOCKS[g]
                tTg = []
                for q in range(KQ):
                    tt = tT_pool.tile([128, 2 * bsz], FP8, name="tT",
                                      tag="tT", bufs=4,
                                      padded_shape=[128, 2 * BLK])
                    ttu = tt.bitcast(U16).rearrange("p (i n) -> p i n", i=2)
                    for i in range(2):
                        qt = 2 * q + i
                        nc.sync.dma_start_transpose(
                            ttu[:, i:i + 1, :],
                            t8u[goff // 2:(goff + bsz) // 2,
                                qt * 128:(qt + 1) * 128])
                    tTg.append(tt)
                return tTg

            def block_matmul(g, tTg):
                goff, bsz = BLOCKS[g]
                for m in range(MT):
                    ps = psum_pool.tile([128, bsz], F32, name="ps", tag="ps",
                                        padded_shape=[128, BLK])
                    for q in range(KQ):
                        w_ap = pw[q].rearrange(
                            "j (i m) -> j i m",
                            i=2)[:, :, 128 * m:128 * (m + 1)]
                        x3 = tTg[q].rearrange("j (i n) -> j i n", i=2)
                        for j in range(bsz // 512):
                            nc.tensor.matmul(
                                ps[:, j * 512:(j + 1) * 512], w_ap,
                                x3[:, :, j * 512:(j + 1) * 512],
                                start=(q == 0), stop=(q == KQ - 1),
                                perf_mode=PM.DoubleRow)
                    nc.scalar.activation(
                        ps[:], ps[:], AF.Exp, scale=expsc[:, m:m + 1],
                        accum_out=sume[:, m * NBLK + g:m * NBLK + g + 1])

            # ---------------- Phase A/B interleaved pipeline ----------------
            # loads: block 0 first (critical path), then pred/td, then more
            load_block(0)
            pq, tdq = [], []
            for k in range(M_LOCAL // QROWS):
                p_ld = ld_pool.tile([128, 4 * C], F32, name=f"pld{k}",
                                    tag=f"pld{k}")
                nc.sync.dma_start(p_ld[:].rearrange("p (q c) -> p q c", c=C),
                                  quad_dram_ap(pred, k * QROWS, QROWS))
                t_ld = ld_pool.tile([128, 4 * C], F32, name=f"tdld{k}",
                                    tag=f"tdld{k}")
                nc.sync.dma_start(t_ld[:].rearrange("p (q c) -> p q c", c=C),
                                  quad_dram_ap(td, k * QROWS, QROWS))
                pq.append(p_ld)
                tdq.append(t_ld)
            load_block(1)
            load_block(2)
            load_block(3)

            # block 0 norms + rsqrt (first table pair), cast, bounce
            stt0 = block_norms(0)
            rt0 = rsqrt_group([(0, stt0, 4)])
            block_cast_bounce(0, rt0[0])

            # pred bf16 cast (DVE) + bounce
            for k in range(M_LOCAL // QROWS):
                pbq = q8_pool.tile([128, 4 * C], BF16, name=f"pbq{k}",
                                   tag=f"pbq{k}")
                for s in range(4):
                    nc.vector.tensor_scalar_mul(
                        pbq[:, s * C:(s + 1) * C],
                        pq[k][:, s * C:(s + 1) * C], 1.0)
                nc.gpsimd.dma_start(quad_dram_ap(pb_dram, k * QROWS, QROWS),
                                    pbq[:].rearrange("p (q c) -> p q c", c=C))

            stt1 = block_norms(1)
            sp = stats_pool.tile([128, MT], F32, name="sp", tag="sp")
            std = stats_pool.tile([128, MT], F32, name="std", tag="std")
            d0 = stats_pool.tile([128, MT], F32, name="d0", tag="d0")
            for k in range(M_LOCAL // QROWS):
                for s in range(4):
                    col = k * 4 + s
                    a = pq[k][:, s * C:(s + 1) * C]
                    b = tdq[k][:, s * C:(s + 1) * C]
                    sqa = sq_pool.tile([128, C], F32, name="sqa", tag="sqd")
                    nc.vector.scalar_tensor_tensor(
                        sqa[:], a, 1.0, a, ALU.mult, ALU.mult,
                        accum_out=sp[:, col:col + 1])
                    sqb = sq_pool.tile([128, C], F32, name="sqb", tag="sqd")
                    nc.vector.scalar_tensor_tensor(
                        sqb[:], b, 1.0, b, ALU.mult, ALU.mult,
                        accum_out=std[:, col:col + 1])

            # grouped rsqrt for (block1, pred, td): one table swap pair
            rg = rsqrt_group([(1, stt1, 4), ("p", sp, MT), ("td", std, MT)])
            rtt1, rp, rtd = rg[1], rg["p"], rg["td"]

            expsc = stats_pool.tile([128, MT], F32, name="expsc", tag="expsc")
            nc.vector.tensor_scalar_mul(expsc[:], rp[:], SCALE / FP8_GAIN)
            block_cast_bounce(1, rtt1)

            # pred transposes -> planar fp8 weights (scalar Copy casts to
            # keep the DVE queue free for the target pipeline)
            pTb = []
            for kc in range(C // 128):
                pt = pT_pool.tile([128, M_LOCAL], BF16, name=f"pTb{kc}",
                                  tag=f"pTb{kc}")
                nc.sync.dma_start_transpose(
                    pt[:], pb_dram[0:M_LOCAL, kc * 128:(kc + 1) * 128])
                pTb.append(pt)
            pw = []
            for q in range(KQ):
                w = pT_pool.tile([128, 2 * M_LOCAL], FP8, name=f"pw{q}",
                                 tag=f"pw{q}")
                for i in range(2):
                    nc.scalar.copy(w[:, i * M_LOCAL:(i + 1) * M_LOCAL],
                                   pTb[2 * q + i][:])
                pw.append(w)

            tT0 = block_transpose(0)
            block_matmul(0, tT0)

            stt2 = block_norms(2)
            stt3 = block_norms(3)
            rg23 = rsqrt_group([(2, stt2, 8), (3, stt3, 16)])
            block_cast_bounce(2, rg23[2])
            tT1 = block_transpose(1)
            block_matmul(1, tT1)

            load_block(4)
            block_cast_bounce(3, rg23[3])
            tT2 = block_transpose(2)
            block_matmul(2, tT2)

            # deferred diag dot products (DVE idle window), then diag scale
            for k in range(M_LOCAL // QROWS):
                for s in range(4):
                    col = k * 4 + s
                    a = pq[k][:, s * C:(s + 1) * C]
                    b = tdq[k][:, s * C:(s + 1) * C]
                    sqc = sq_pool.tile([128, C], F32, name="sqc", tag="sqd")
                    nc.vector.scalar_tensor_tensor(
                        sqc[:], a, 1.0, b, ALU.mult, ALU.mult,
                        accum_out=d0[:, col:col + 1])
            dtmp = stats_pool.tile([128, MT], F32, name="dtmp", tag="dtmp")
            nc.vector.tensor_mul(dtmp[:], d0[:], rtd[:])
            diag = stats_pool.tile([128, MT], F32, name="diag", tag="diag")
            nc.vector.scalar_tensor_tensor(
                diag[:], dtmp[:], SCALE, rp[:], ALU.mult, ALU.mult)

            load_block(5)
            stt4 = block_norms(4)
            stt5 = block_norms(5)
            rg45 = rsqrt_group([(4, stt4, 16), (5, stt5, 16)])
            block_cast_bounce(4, rg45[4])
            tT3 = block_transpose(3)
            block_matmul(3, tT3)
            block_cast_bounce(5, rg45[5])
            tT4 = block_transpose(4)
            block_matmul(4, tT4)
            tT5 = block_transpose(5)
            block_matmul(5, tT5)

            # ---------------- Phase C: lse - diag --------------------------
            rowsum = stats_pool.tile([128, MT], F32, name="rowsum",
                                     tag="rowsum")
            nc.vector.tensor_reduce(
                rowsum[:], sume[:].rearrange("p (m g) -> p m g", g=NBLK),
                axis=AXIS.X, op=ALU.add)
            lse = stats_pool.tile([128, MT], F32, name="lse", tag="lse")
            nc.scalar.activation(lse[:], rowsum[:], AF.Ln)
            losst = stats_pool.tile([128, MT], F32, name="losst", tag="losst")
            nc.vector.tensor_sub(losst[:], lse[:], diag[:])
            nc.sync.dma_start(out[:], losst[:])

    nc.compile()
    return nc


_NC_CACHE = {}


def _get_nc():
    key = (M_LOCAL, N_TOTAL, C)
    if key not in _NC_CACHE:
        _NC_CACHE[key] = build_nc()
    return _NC_CACHE[key]


def run_cores(pred2d, tgt2d, trace=False):
    """Run the SPMD program on cores 0..7; returns (partials [8,128,MT], res)."""
    nc = _get_nc()
    in_maps = []
    for ci in range(N_CORES):
        r0 = ci * M_LOCAL
        in_maps.append({
            "pred": np.ascontiguousarray(pred2d[r0:r0 + M_LOCAL]),
            "tgt": np.ascontiguousarray(tgt2d),
            "td": np.ascontiguousarray(tgt2d[r0:r0 + M_LOCAL]),
        })
    res = run_bass_kernel_spmd(nc, in_maps, list(range(N_CORES)), trace=trace)
    partials = np.stack([res.results[i]["out"] for i in range(N_CORES)])
    return partials, res


def kernel(pred, target):
    pred2d = np.asarray(pred, dtype=np.float32).reshape(-1, C)
    tgt2d = np.asarray(target, dtype=np.float32).reshape(-1, C)
    partials, _ = run_cores(pred2d, tgt2d)
    loss = partials.astype(np.float64).sum() / float(N_TOTAL)
    return np.float32(loss)
